# revision 6
# baseline (speedup 1.0000x reference)
"""Trainium2 Bass kernel for BigVGAN AMPBlock1 (nn_AMPBlock1_81655918231624).

Strategy: data-parallel over batch (B=8 -> 1 sample per NeuronCore).
Per core, the whole block runs channel-major ([128 part = channel mod 128,
4 groups, T]) as 6 sequential "units" (act1d + dilated conv), with DRAM
spill between units:

  - up/down anti-alias FIRs: polyphase 6-tap filters as diagonal-matrix
    f32r matmuls on the TensorEngine (PSUM-accumulated).
  - snakebeta: x + sin^2(a*x)/b via range-reduced Sin on ScalarE:
      q = u * a/(2pi)            (DVE, PSUM read)
      r = (q + M) - M            (DVE, fp32 magic-number round)
      f = q - r  in [-0.5, 0.5]  (DVE)
      s = Sin(2pi * f)           (ScalarE; = +-sin(a*u), sign dies in square)
      p = Square(s * sqrt(1/b))  (ScalarE; = sin^2(a*u)/b)
      z = u + p                  (DVE)
  - 512x512 k=3 dilated convs: f32r matmuls, 4x4 channel blocks x 3 taps.
  - residual adds fused into the conv2 PSUM eviction.

Self-contained: shapes hardcoded; no sibling imports.
"""
import numpy as np

# ---------------------------------------------------------------- constants
B, C, T = 8, 512, 8192
G, P = 4, 128            # channel groups x partitions
KER = 3
DILATIONS = (1, 3, 5)
FILT_K = 12
TC = 256                 # output columns per time-tile
NTILES = T // TC
MAGIC = 12582912.0       # 1.5 * 2**23: fp32 round-to-nearest-int
N_CORES = 8


def _kaiser_sinc_filter1d(cutoff, half_width, kernel_size):
    even = kernel_size % 2 == 0
    half_size = kernel_size // 2
    delta_f = 4 * half_width
    A = 2.285 * (half_size - 1) * np.pi * delta_f + 7.95
    if A > 50.0:
        beta = 0.1102 * (A - 8.7)
    elif A >= 21.0:
        beta = 0.5842 * (A - 21) ** 0.4 + 0.07886 * (A - 21.0)
    else:
        beta = 0.0
    window = np.kaiser(kernel_size, beta)
    if even:
        time = np.arange(-half_size, half_size) + 0.5
    else:
        time = np.arange(kernel_size) - half_size
    if cutoff == 0:
        filt = np.zeros(kernel_size)
    else:
        filt = 2 * cutoff * window * np.sinc(2 * cutoff * time)
        filt = filt / np.sum(filt)
    return filt


def _polyphase_filters():
    """up even: u[2t]   = sum_i fe[i] * xc[t-3+i]
       up odd:  u[2t+1] = sum_i fo[i] * xc[t-2+i]
       down:    y[t] = sum_j fde[j]*ze[t-2+j] + fdo[j]*zo[t-3+j]"""
    up = _kaiser_sinc_filter1d(0.25, 0.3, FILT_K)
    dn = _kaiser_sinc_filter1d(0.25, 0.3, FILT_K)
    wf = 2.0 * up[::-1]
    return wf[0::2], wf[1::2], dn[1::2], dn[0::2]


def _build_nc(has_bias, has_act):
    """Builds the Bacc graph. has_bias/has_act: enable general paths."""
    from concourse import bacc, mybir, tile

    f32r = mybir.dt.float32r
    f32 = mybir.dt.float32
    AF = mybir.ActivationFunctionType
    ALU = mybir.AluOpType

    nc = bacc.Bacc("TRN2", target_bir_lowering=False, debug=False,
                   num_devices=N_CORES)

    x_ext = nc.dram_tensor("x", [P, G, T], f32r, kind="ExternalInput").ap()
    # weights: [unit, part=ci%128, k, gi, co(512)]
    w_ext = nc.dram_tensor("w", [6, P, KER, G, C], f32r,
                           kind="ExternalInput").ap()
    # 24 diagonal filter matrices, partition-major: [part, idx, 128]
    dg_ext = nc.dram_tensor("dg", [P, 24, P], f32r, kind="ExternalInput").ap()
    # per-act scalars: [part, unit, g, {a/(2pi), sqrt(1/(b+eps))}]
    sc_ext = nc.dram_tensor("sc", [P, 6, G, 2], f32, kind="ExternalInput").ap()
    bias_ext = nc.dram_tensor("bias", [P, 6, G], f32, kind="ExternalInput").ap()
    out_ext = nc.dram_tensor("out", [P, G, T], f32, kind="ExternalOutput").ap()

    spill = [nc.dram_tensor(f"spill{j}", [P, G, T], f32r, kind="Internal").ap()
             for j in range(5)]
    # unit j: input src, output dst, residual (None if no add)
    unit_src = [x_ext, spill[0], spill[1], spill[2], spill[3], spill[4]]
    unit_dst = [spill[0], spill[1], spill[2], spill[3], spill[4], out_ext]
    unit_res = [None, x_ext, None, spill[1], None, spill[3]]

    with tile.TileContext(nc) as tc:
        with tc.tile_pool(name="const", bufs=1) as cpool, \
             tc.tile_pool(name="wpool", bufs=2) as wpool, \
             tc.tile_pool(name="sbuf", bufs=3) as pool, \
             tc.tile_pool(name="psum", bufs=1, space="PSUM") as psp:

            dg = cpool.tile([P, 24, P], f32r)
            nc.sync.dma_start(dg[:], dg_ext[:])
            sc = cpool.tile([P, 6, G, 2], f32)
            nc.sync.dma_start(sc[:], sc_ext[:])
            bias_t = cpool.tile([P, 6, G], f32)
            if has_bias:
                nc.sync.dma_start(bias_t[:], bias_ext[:])

            for j in range(6):
                d = DILATIONS[j // 2] if j % 2 == 0 else 1
                h = d + 6                 # left halo in x (col0 = t0 - h)
                XL = TC + 2 * d + 12      # x tile length
                L = TC + 2 * d + 6        # phase (u/z) length, even
                SA = TC + 2 * d           # act output length
                src, dst, res = unit_src[j], unit_dst[j], unit_res[j]

                wt = wpool.tile([P, KER, G, C], f32r, name=f"wt{j}", tag="wt")
                nc.sync.dma_start(wt[:], w_ext[j])

                for i in range(NTILES):
                    t0 = i * TC
                    lo = t0 - h              # absolute x index of x_in col 0
                    x_in = pool.tile([P, G, XL], f32r, name=f"xin{j}_{i}",
                                     tag="xin")
                    # ---- input DMA with edge clamping
                    lo_c = max(lo, 0)
                    hi_c = min(lo + XL, T)
                    nc.sync.dma_start(x_in[:, :, lo_c - lo:hi_c - lo],
                                      src[:, :, lo_c:hi_c])
                    for c in range(lo_c - lo):                    # left clamp
                        nc.sync.dma_start(x_in[:, :, c:c + 1], src[:, :, 0:1])
                    for c in range(hi_c - lo, XL):               # right clamp
                        nc.sync.dma_start(x_in[:, :, c:c + 1],
                                          src[:, :, T - 1:T])

                    if res is not None:
                        res_t = pool.tile([P, G, TC], f32r,
                                          name=f"res{j}_{i}", tag="res")
                        nc.sync.dma_start(res_t[:], res[:, :, t0:t0 + TC])

                    # ---- act1d: up (diag matmuls) + snake + down
                    z_ph = []
                    for ph, base in ((0, 0), (1, 6)):
                        z_t = pool.tile([P, G, L], f32r,
                                        name=f"z{j}_{i}_{ph}", tag=f"z{ph}")
                        for g in range(G):
                            pu = psp.tile([P, L], f32, name=f"pu{j}_{i}_{ph}_{g}",
                                          tag="pu", bufs=3)
                            for ii in range(6):
                                nc.tensor.matmul(pu[:], dg[:, base + ii, :],
                                                 x_in[:, g, ii + 1:ii + 1 + L],
                                                 start=(ii == 0), stop=(ii == 5))
                            q_t = pool.tile([P, L], f32, name=f"q{j}_{i}_{ph}_{g}",
                                            tag="q", bufs=2)
                            nc.vector.tensor_scalar_mul(q_t[:], pu[:],
                                                        sc[:, j, g, 0:1])
                            r_t = pool.tile([P, L], f32, name=f"r{j}_{i}_{ph}_{g}",
                                            tag="r", bufs=2)
                            nc.vector.tensor_scalar(r_t[:], q_t[:], MAGIC, MAGIC,
                                                    op0=ALU.add, op1=ALU.subtract)
                            f_t = pool.tile([P, L], f32, name=f"f{j}_{i}_{ph}_{g}",
                                            tag="f", bufs=2)
                            nc.vector.tensor_sub(f_t[:], q_t[:], r_t[:])
                            s_t = pool.tile([P, L], f32, name=f"s{j}_{i}_{ph}_{g}",
                                            tag="s", bufs=2)
                            nc.scalar.activation(s_t[:], f_t[:], AF.Sin,
                                                 bias=0.0, scale=float(2 * np.pi))
                            p_t = pool.tile([P, L], f32, name=f"p{j}_{i}_{ph}_{g}",
                                            tag="p", bufs=2)
                            nc.scalar.activation(p_t[:], s_t[:], AF.Square,
                                                 bias=0.0, scale=sc[:, j, g, 1:2])
                            nc.vector.tensor_add(z_t[:, g, :], pu[:], p_t[:])
                        z_ph.append(z_t)
                    z_e, z_o = z_ph

                    # ---- z edge clamping (replicate-pad semantics of down)
                    # z_e col c is z-phase-e index mE + c, mE = t0 - d - 2
                    # z_o col c is z-phase-o index mO + c, mO = t0 - d - 3
                    mE = t0 - d - 2
                    mO = t0 - d - 3
                    if i == 0:
                        srcc = -mE        # col of z_e[m=0]
                        for c in range(-mE):          # z_e[m<0] = z_e[0]
                            nc.vector.tensor_copy(z_e[:, :, c:c + 1],
                                                  z_e[:, :, srcc:srcc + 1])
                        for c in range(-mO):          # z_o[m<0] = z_e[0]
                            nc.vector.tensor_copy(z_o[:, :, c:c + 1],
                                                  z_e[:, :, srcc:srcc + 1])
                    if i == NTILES - 1:
                        srco = T - 1 - mO  # col of z_o[m=T-1]
                        for c in range(T - mE, L):    # z_e[m>=T] = z_o[T-1]
                            nc.vector.tensor_copy(z_e[:, :, c:c + 1],
                                                  z_o[:, :, srco:srco + 1])
                        for c in range(T - mO, L):    # z_o[m>=T] = z_o[T-1]
                            nc.vector.tensor_copy(z_o[:, :, c:c + 1],
                                                  z_o[:, :, srco:srco + 1])

                    y_act = pool.tile([P, G, SA], f32r, name=f"ya{j}_{i}",
                                      tag="ya")
                    for g in range(G):
                        pd = psp.tile([P, SA], f32, name=f"pd{j}_{i}_{g}",
                                      tag="pd", bufs=2)
                        for jj in range(6):
                            nc.tensor.matmul(pd[:], dg[:, 12 + jj, :],
                                             z_e[:, g, jj:jj + SA],
                                             start=(jj == 0), stop=False)
                        for jj in range(6):
                            nc.tensor.matmul(pd[:], dg[:, 18 + jj, :],
                                             z_o[:, g, jj:jj + SA],
                                             start=False, stop=(jj == 5))
                        nc.scalar.activation(y_act[:, g, :], pd[:], AF.Copy)

                    # conv zero-padding: act output t<0 or t>=T must be 0
                    if i == 0 and d > 0:
                        nc.vector.memset(y_act[:, :, 0:d].bitcast(f32), 0.0)
                    if i == NTILES - 1 and d > 0:
                        nc.vector.memset(y_act[:, :, SA - d:SA].bitcast(f32), 0.0)

                    # ---- dilated conv 512x512 k=3
                    out_t = pool.tile([P, G, TC], f32r if j < 5 else f32,
                                      name=f"ot{j}_{i}", tag="ot")
                    for go in range(G):
                        pc = psp.tile([P, TC], f32, name=f"pc{j}_{i}_{go}",
                                      tag="pc", bufs=2)
                        first = True
                        for k in range(KER):
                            for gi in range(G):
                                nc.tensor.matmul(
                                    pc[:], wt[:, k, gi, go * P:(go + 1) * P],
                                    y_act[:, gi, k * d:k * d + TC],
                                    start=first, stop=(k == KER - 1 and gi == G - 1))
                                first = False
                        if res is not None:
                            if has_bias:
                                tmp = pool.tile([P, TC], f32, name=f"tb{j}_{i}_{go}",
                                                tag="tb", bufs=2)
                                nc.scalar.activation(tmp[:], pc[:], AF.Identity,
                                                     bias=bias_t[:, j, go:go + 1])
                                nc.vector.tensor_add(
                                    out_t[:, go, :], tmp[:],
                                    res_t[:, go, :].bitcast(f32))
                            else:
                                nc.vector.tensor_add(
                                    out_t[:, go, :], pc[:],
                                    res_t[:, go, :].bitcast(f32))
                        else:
                            if has_bias:
                                nc.scalar.activation(out_t[:, go, :], pc[:],
                                                     AF.Identity,
                                                     bias=bias_t[:, j, go:go + 1])
                            else:
                                nc.scalar.activation(out_t[:, go, :], pc[:],
                                                     AF.Copy)
                    nc.sync.dma_start(dst[:, :, t0:t0 + TC], out_t[:])
    nc.compile()
    return nc


_NC_CACHE = {}
LAST_EXEC_NS = None


def kernel(**inputs):
    from concourse.bass_utils import run_bass_kernel_spmd

    x = np.asarray(inputs["x"], dtype=np.float32)          # [B, C, T]
    fe, fo, fde, fdo = _polyphase_filters()

    # diag filter matrices [P, 24, P]
    dg = np.zeros((P, 24, P), dtype=np.float32)
    coeffs = list(fe) + list(fo) + list(fde) + list(fdo)
    for idx, cf in enumerate(coeffs):
        np.fill_diagonal(dg[:, idx, :], cf)

    # weights [6, P, KER, G, C]: unit 2l -> w1_l, unit 2l+1 -> w2_l
    w_all = np.zeros((6, P, KER, G, C), dtype=np.float32)
    b_all = np.zeros((P, 6, G), dtype=np.float32)
    for l in range(3):
        for half, nm in ((0, "w1"), (1, "w2")):
            j = 2 * l + half
            w = np.asarray(inputs[f"{nm}_{l}"], dtype=np.float32)  # [co, ci, k]
            # -> [ci%128, k, gi, co]
            w_all[j] = w.transpose(1, 2, 0).reshape(G, P, KER, C).transpose(
                1, 2, 0, 3)
            b = np.asarray(inputs[f"b{half + 1}_{l}"], dtype=np.float32)
            b_all[:, j, :] = b.reshape(G, P).T

    sc = np.zeros((P, 6, G, 2), dtype=np.float32)
    for j in range(6):
        a = np.exp(np.asarray(inputs[f"alpha_{j}"], dtype=np.float64))
        bb = np.exp(np.asarray(inputs[f"beta_{j}"], dtype=np.float64))
        sc[:, j, :, 0] = (a / (2 * np.pi)).reshape(G, P).T
        sc[:, j, :, 1] = (1.0 / np.sqrt(bb + 1e-9)).reshape(G, P).T

    has_bias = bool(np.any(b_all != 0.0))
    key = has_bias
    if key not in _NC_CACHE:
        _NC_CACHE[key] = _build_nc(has_bias, True)
    nc = _NC_CACHE[key]

    in_maps = []
    for bi in range(B):
        xb = x[bi].reshape(G, P, T).transpose(1, 0, 2).copy()  # [P, G, T]
        in_maps.append({"x": xb, "w": w_all, "dg": dg, "sc": sc,
                        "bias": b_all})
    res = run_bass_kernel_spmd(nc, in_maps, core_ids=list(range(N_CORES)))
    global LAST_EXEC_NS
    LAST_EXEC_NS = res.exec_time_ns
    out = np.empty((B, C, T), dtype=np.float32)
    for bi in range(B):
        ob = res.results[bi]["out"]                      # [P, G, T]
        out[bi] = ob.transpose(1, 0, 2).reshape(C, T)
    return out
